# revision 41
# baseline (speedup 1.0000x reference)
"""Trainium2 Bass kernel for MemoryEfficientFlashAttention (B=2,S=2048,HID=2048,H=16,HKV=8,D=128,CHUNK=512).

Sharding: 8 cores = 2 batches x 4 head-groups (4 q heads / 2 kv heads per core).
Each core computes q/k/v projections (+RoPE), the chunked flash-attention
recurrence, and a row-sharded partial of the output projection (transposed).
Host sums the 4 partials per batch and adds bo.

Math: the reference's scan step is algebraically
    o_j = (o_{j-1} * e^{m_{j-1}} + Y_j) / (e^{m_{j-1}} + S_j)
with Y_j = exp(sc_j) @ V_j, S_j = rowsum exp(sc_j), m_j = running max.
Unrolled:  o_n = sum_j Y_j * C_{j-1} / (C_n * e^{m_n}),  C_j = prod_{l<=j} d_l,
    d_l = e^{m_{l-1}-m_l} + T_l,  T_l = rowsum exp(sc_l - m_l).
Pass 1 computes the (m, T, d, lnC) chains per row; pass 2 recomputes scores
transposed and accumulates  u = sum_j exp(sc_j^T + w_j - gamma) @ V  directly
in PSUM, with w_j = lnC_{j-1} and gamma = m_n + lnC_n (+ ln d_n if the
globally-last kv chunk was processed, reproducing the reference's final o/d
divide).  u is then exactly the final attention output; exponents are <= 0 so
everything is numerically stable.

Perf structure: bf16 operands for all large matmuls (full-rate at any moving
width), causal narrowing of the diagonal chunks (skip fully-masked k/q
sub-ranges), a single shared 128x128 triangular mask tile instead of
per-block mask DMA, single f32r rank-1 inject for the per-chunk log-scale
w, weights resident in SBUF (loaded once), and pass-1 (Act/DVE-heavy)
interleaved with the projections (PE-heavy).
"""

import os
import sys
from contextlib import ExitStack

import numpy as np
import ml_dtypes

sys.path.insert(0, "/opt/trn_rl_repo")
os.environ.setdefault("MYCRO_LOCAL_CACHE", "1")

import concourse.bass as bass  # noqa: E402
import concourse.tile as tile  # noqa: E402
from concourse import bacc, mybir  # noqa: E402
from concourse.bass_utils import run_bass_kernel_spmd  # noqa: E402

# Steer insert_act_table_loads to the table set that holds BOTH Exp and Ln
# (natural_log_exp_and_others) so the kernel loads one activation table
# instead of thrashing Exp<->Ln sets per query chunk. Indices into the
# act_info.json list are preserved; only the selection sees fewer options.
import collections  # noqa: E402
import concourse.hw_specs as _hw_specs  # noqa: E402

_gat_orig = _hw_specs.get_activation_tables


def _gat_combined(arch):
    tabs = _gat_orig(arch)
    both = {mybir.ActivationFunctionType.Exp, mybir.ActivationFunctionType.Ln}
    out = collections.OrderedDict()
    for name, s in tabs.items():
        if name == "natural_log_exp_and_others" or not (s & both):
            out[name] = s
        else:
            out[name] = s - both
    return out


bacc.get_activation_tables = _gat_combined

B, S, HID = 2, 2048, 2048
H, HKV, D = 16, 8, 128
CHUNK = 512
THETA = 1000000.0
NEG = -1e9
NCORES = 8
HL = H // (NCORES // B)      # 4 local q heads
KVL = HKV // (NCORES // B)   # 2 local kv heads
NQ = S // CHUNK              # 4 chunks
NT = HID // 128              # 16 hid tiles
SCALE = 1.0 / np.sqrt(np.float32(D))

F32 = mybir.dt.float32
F32R = mybir.dt.float32r
BF16 = mybir.dt.bfloat16
Alu = mybir.AluOpType
Act = mybir.ActivationFunctionType
BFNP = ml_dtypes.bfloat16

_CACHE = {}


def _rope_tables():
    inv_freq = 1.0 / (THETA ** (np.arange(0, D, 2, dtype=np.float32) / D))
    pos = np.arange(S, dtype=np.float32)
    freqs = pos[:, None].astype(np.float32) * inv_freq[None, :]
    emb = np.concatenate([freqs, freqs], axis=-1)  # [S, D]
    cosT = np.cos(emb).astype(np.float32).T.copy()
    sinT = np.sin(emb).astype(np.float32).T.copy()
    return cosT, sinT  # [D, S]


def _classify_mask(attention_mask):
    """Per (qi, j) CHUNKxCHUNK block: 'zero' | 'neg' | 'tri' (canonical causal
    diagonal), merged across batches so the SPMD program is identical on all
    cores. Only pure-causal masks are supported by this kernel."""
    q = np.arange(CHUNK)
    tri_full = np.where(q[:, None] >= q[None, :], 0.0, NEG).astype(np.float32)
    kinds = {}
    for qi in range(NQ):
        for j in range(NQ):
            kind = None
            for b in range(B):
                blk = attention_mask[b, 0, qi * CHUNK:(qi + 1) * CHUNK,
                                     j * CHUNK:(j + 1) * CHUNK]
                if np.all(blk == 0.0):
                    k = "zero"
                elif np.all(blk <= -1e6):
                    k = "neg"
                elif np.array_equal(blk, tri_full):
                    k = "tri"
                else:
                    raise NotImplementedError("non-causal mask block")
                if kind is None:
                    kind = k
                elif kind != k:
                    raise NotImplementedError("mask differs across batches")
            kinds[(qi, j)] = kind
    plan = {}
    for qi in range(NQ):
        processed = []
        for j in range(NQ):
            k = kinds[(qi, j)]
            if k == "neg" and len(processed) > 0:
                continue  # identity step under the reference's fp32 exp underflow
            assert k != "neg" or len(processed) == 0
            if k == "neg":
                # leading fully-masked chunk: contributes T=0 rows; unsupported
                raise NotImplementedError("leading all-neg chunk")
            processed.append((j, k == "tri"))
        plan[qi] = processed
    return plan


def _mm(nc, out, lhsT, rhs, start, stop):
    nc.tensor.matmul(out, lhsT, rhs, start=start, stop=stop)


def _emit(tc, ap, plan):
    nc = tc.nc

    with ExitStack() as top:
        # ---------------- persistent tensors ----------------
        pers = top.enter_context(tc.tile_pool(name="pers", bufs=1))
        QT = pers.tile([128, HL, S], BF16)             # rope'd q^T  [d, h, s]
        KT = pers.tile([128, KVL, S], BF16)            # rope'd k^T  [d, kv, s]
        V = pers.tile([128, S // 128, KVL * D], BF16)  # v natural [s_p, s_t, kv*d]
        xt_pool = top.enter_context(tc.tile_pool(name="xt", bufs=2))
        hsT_r = ap["hsT"].rearrange("(t p) s -> p t s", p=128)

        xts = {}

        def load_xt(sq):
            xt = xt_pool.tile([128, NT, CHUNK], BF16, tag="xt")
            ssl = slice(sq * CHUNK, (sq + 1) * CHUNK)
            for tq in range(4):
                nc.sync.dma_start(xt[:, tq * 4:(tq + 1) * 4, :],
                                  hsT_r[:, tq * 4:(tq + 1) * 4, ssl])
            xts[sq] = xt

        # startup DMAs ordered by first use: first-half weights + first x
        # chunk + rope tables first, everything else behind them
        wqk_sb = pers.tile([128, NT, (HL + KVL) * 128], BF16)
        wqk_r = ap["wqk"].rearrange("(t p) m -> p t m", p=128)
        ssl0 = slice(0, CHUNK)
        xt0 = xt_pool.tile([128, NT, CHUNK], BF16, tag="xt")
        xts[0] = xt0
        nc.sync.dma_start(wqk_sb[:, :2], wqk_r[:, :2])
        nc.sync.dma_start(xt0[:, :2, :], hsT_r[:, :2, ssl0])
        nc.sync.dma_start(wqk_sb[:, 2:4], wqk_r[:, 2:4])
        nc.sync.dma_start(xt0[:, 2:4, :], hsT_r[:, 2:4, ssl0])
        for tq in range(1, 4):
            nc.sync.dma_start(wqk_sb[:, tq * 4:(tq + 1) * 4],
                              wqk_r[:, tq * 4:(tq + 1) * 4])
            nc.sync.dma_start(xt0[:, tq * 4:(tq + 1) * 4, :],
                              hsT_r[:, tq * 4:(tq + 1) * 4, ssl0])
        cosT = pers.tile([128, S], BF16)
        sinT = pers.tile([128, S], BF16)
        nc.sync.dma_start(cosT[:, ssl0], ap["cosT"][:, ssl0])
        nc.sync.dma_start(sinT[:, ssl0], ap["sinT"][:, ssl0])
        R128 = pers.tile([128, 128], F32R)
        nc.sync.dma_start(R128, ap["rmat"])
        bqk = pers.tile([128, HL + KVL], F32)
        nc.sync.dma_start(bqk, ap["bqk"])
        for cq in range(1, NQ):
            cs = slice(cq * CHUNK, (cq + 1) * CHUNK)
            nc.sync.dma_start(cosT[:, cs], ap["cosT"][:, cs])
            nc.sync.dma_start(sinT[:, cs], ap["sinT"][:, cs])
        wv_sb = pers.tile([128, NT, KVL * D], BF16)
        nc.sync.dma_start(wv_sb, ap["wv"].rearrange("(t p) m -> p t m", p=128))
        bv = pers.tile([1, KVL * D], F32R)
        nc.sync.dma_start(bv, ap["bv"])
        ones1 = pers.tile([1, 128], F32R)
        nc.sync.dma_start(ones1, ap["ones1"])
        ones65 = pers.tile([65, 128], F32R)
        nc.sync.dma_start(ones65, ap["ones65"])
        I128f = pers.tile([128, 128], F32)
        nc.sync.dma_start(I128f, ap["imat"])
        I128b = pers.tile([128, 128], BF16)
        nc.sync.dma_start(I128b, ap["imatb"])
        triN = pers.tile([128, 128], BF16)
        nc.sync.dma_start(triN, ap["triN"])
        triT = pers.tile([128, 128], BF16)
        nc.sync.dma_start(triT, ap["triT"])
        wo_sb = pers.tile([128, HL, HID], BF16)
        wo_r = ap["wo"].rearrange("(t p) m -> p t m", p=128)
        for mo in range(4):
            nc.sync.dma_start(wo_sb[:, :, mo * 512:(mo + 1) * 512],
                              wo_r[:, :, mo * 512:(mo + 1) * 512])

        # ---------------- pools (single scope; PSUM budget = 8 banks) ------
        raw_pool = top.enter_context(tc.tile_pool(name="raw", bufs=2))
        t_pool = top.enter_context(tc.tile_pool(name="ropetmp", bufs=2))
        ps_proj = top.enter_context(tc.tile_pool(name="psproj", bufs=3, space="PSUM"))
        ps_att = top.enter_context(tc.tile_pool(name="psatt", bufs=4, space="PSUM"))
        u_ps = top.enter_context(tc.tile_pool(name="ups", bufs=1, space="PSUM"))

        ch_pool = top.enter_context(tc.tile_pool(name="chain", bufs=2))
        ws_pool = top.enter_context(tc.tile_pool(name="wstar", bufs=1))
        scr_pool = top.enter_context(tc.tile_pool(name="scratch", bufs=3))
        wt2_pool = top.enter_context(tc.tile_pool(name="wt2p", bufs=1))
        wf_pool = top.enter_context(tc.tile_pool(name="wflat", bufs=1))
        p2_pool = top.enter_context(tc.tile_pool(name="pprime", bufs=5))
        o2_pool = top.enter_context(tc.tile_pool(name="uout", bufs=2))
        o_pool = top.enter_context(tc.tile_pool(name="osb", bufs=4))

        wt_tiles = {}

        def proj_qk(sq):
            ssl = slice(sq * CHUNK, (sq + 1) * CHUNK)
            xt = xts.pop(sq)
            if sq + 1 < NQ:
                load_xt(sq + 1)

            # q^T and k^T projections, rope'd; the R-matmul + elementwise
            # rope tail run one m behind the qk accumulation so the PE never
            # waits on the Pool-engine bias add
            def rope_tail(m, raw):
                pr = ps_proj.tile([128, CHUNK], F32, tag="pp")
                _mm(nc, pr, R128, raw, start=True, stop=True)
                t1 = t_pool.tile([128, CHUNK], F32, tag="t1")
                nc.gpsimd.tensor_mul(t1, raw.bitcast(F32), cosT[:, ssl])
                t2 = t_pool.tile([128, CHUNK], F32, tag="t2")
                nc.vector.tensor_mul(t2, pr, sinT[:, ssl])
                dest = QT[:, m, ssl] if m < HL else KT[:, m - HL, ssl]
                nc.vector.tensor_add(dest, t1, t2)

            pend_rope = []
            for m in range(HL + KVL):
                ps = ps_proj.tile([128, CHUNK], F32, tag="pp")
                for t in range(NT):
                    _mm(nc, ps, wqk_sb[:, t, m * 128:(m + 1) * 128], xt[:, t],
                        start=(t == 0), stop=(t == NT - 1))
                raw = raw_pool.tile([128, CHUNK], F32R)
                nc.vector.tensor_scalar_add(raw, ps, bqk[:, m:m + 1])
                pend_rope.append((m, raw))
                if len(pend_rope) > 1:
                    rope_tail(*pend_rope.pop(0))
            for item in pend_rope:
                rope_tail(*item)

        def proj_v(sq):
            # v projection (natural layout), bias via K=1 matmul; runs late
            # (during the Act-bound attention phase) on a reloaded x chunk
            xt = xts.pop(sq)
            for ss in range(CHUNK // 128):
                pv = ps_proj.tile([128, CHUNK], F32, tag="pp")
                for t in range(NT):
                    _mm(nc, pv[:, :KVL * D], xt[:, t, ss * 128:(ss + 1) * 128], wv_sb[:, t],
                        start=(t == 0), stop=False)
                _mm(nc, pv[:, :KVL * D], ones1, bv, start=False, stop=True)
                nc.vector.tensor_copy(V[:, sq * 4 + ss, :], pv[:, :KVL * D])

        def pass1_begin(qi):
            chunks = plan[qi]  # list of (j, is_diag)
            nj = len(chunks)
            # nmstack[:, t, :] = running max BEFORE chunk t (+m domain)
            nms = ws_pool.tile([128, nj + 1, HL * 4], F32, tag=f"nms{qi}")
            nc.vector.memset(nms[:, 0, :], -1e30)
            sraw = ws_pool.tile([128, nj, HL * 4], F32, tag=f"sr{qi}")
            dstore = ws_pool.tile([128, nj, HL * 4], F32, tag=f"ds{qi}")
            return {"qi": qi, "chunks": chunks, "nj": nj, "nms": nms,
                    "sraw": sraw, "dstore": dstore}

        # ---- running max + exp-sum chains (one chunk) ----
        # scores are O(6) here, so exp(sc) cannot overflow: accumulate
        # raw sums S_raw = sum exp(sc) on the Act engine (decoupled from
        # the running-max chain) and rescale T = S_raw * e^{-m} after.
        def pass1_chunk(st, t):
            qi, nms, sraw = st["qi"], st["nms"], st["sraw"]
            for tt, (j, diag) in enumerate(st["chunks"]):
                if tt != t:
                    continue
                k0 = j * CHUNK
                mxe = ch_pool.tile([128, HL * 4], F32, tag="mxe")
                lnmx = ch_pool.tile([128, HL * 4], F32, tag="lnmx")
                for h in range(HL):
                    for sub in range(4):
                        col = h * 4 + sub
                        q0 = qi * CHUNK + sub * 128
                        w = (sub + 1) * 128 if diag else CHUNK
                        ps = ps_att.tile([128, CHUNK], F32, tag="ps")
                        _mm(nc, ps[:, :w], QT[:, h, q0:q0 + 128],
                            KT[:, h // 2, k0:k0 + w],
                            start=True, stop=not diag)
                        if diag:
                            _mm(nc, ps[:, w - 128:w], I128b, triN,
                                start=False, stop=True)
                        scr2 = scr_pool.tile([128, CHUNK], BF16, tag="exp_out")
                        nc.scalar.activation(
                            scr2[:, :w], ps[:, :w], Act.Exp,
                            accum_out=sraw[:, t, col:col + 1])
                        # row max from the bf16 exp output: 2x DVE mode, and
                        # m = ln(max exp(sc)) recovers the running max
                        nc.vector.tensor_reduce(
                            mxe[:, col:col + 1], scr2[:, :w],
                            axis=mybir.AxisListType.X, op=Alu.max)
                nc.scalar.activation(lnmx, mxe, Act.Ln)
                nc.vector.tensor_tensor(nms[:, t + 1, :], nms[:, t, :],
                                        lnmx, Alu.max)

        def pass1_end(st):
            qi, nj, nms, chunks = st["qi"], st["nj"], st["nms"], st["chunks"]
            sraw, dstore = st["sraw"], st["dstore"]
            lnq = ws_pool.tile([128, nj, HL * 4], F32, tag=f"ln{qi}")
            Wadj = ws_pool.tile([128, nj, HL * 4], F32, tag=f"wa{qi}")
            # batched chain tail: T = S_raw * e^{-m_new}, d = e^{m_old-m_new}+T
            dm = ch_pool.tile([128, nj, HL * 4], F32, tag="dmall")
            nc.vector.tensor_sub(dm, nms[:, :nj, :], nms[:, 1:, :])
            pj = ch_pool.tile([128, nj, HL * 4], F32, tag="pjall")
            nc.scalar.activation(pj, dm, Act.Exp)
            emn = ch_pool.tile([128, nj, HL * 4], F32, tag="emnall")
            nc.scalar.activation(emn, nms[:, 1:, :], Act.Exp, scale=-1.0)
            nc.vector.tensor_mul(dstore, sraw, emn)
            nc.vector.tensor_add(dstore, dstore, pj)
            m_fin = nms[:, nj, :]
            # inject_t = -m_n - ln(prod_{l>=t} d_l * d_n^flag): backward
            # products then ONE batched Ln (avoids Exp<->Ln table thrash)
            if any(j == NQ - 1 for (j, _) in chunks):
                nc.vector.tensor_mul(dstore[:, nj - 1, :],
                                     dstore[:, nj - 1, :],
                                     dstore[:, nj - 1, :])
            for t in range(nj - 2, -1, -1):
                nc.vector.tensor_mul(dstore[:, t, :], dstore[:, t, :],
                                     dstore[:, t + 1, :])
            nc.scalar.activation(lnq, dstore, Act.Ln)
            for t in range(nj):
                nc.vector.tensor_add(Wadj[:, t, :], m_fin, lnq[:, t, :])

            # transpose Wadj -> wt2 [nj*HL, 512] (row = (t, h), col = sq),
            # then flatten rows onto partition 0 (matmul rhs needs base
            # partition 0) as f32r for the single rank-1 inject
            wtp = ps_att.tile([nj * HL, 4, 128], F32, tag="ps", name=f"wtp{qi}")
            wadj_r = Wadj.rearrange("p n (x a) -> p n x a", a=4)
            for sub in range(4):
                nc.tensor.transpose(wtp[:, sub, :], wadj_r[:, :, :, sub], I128f)
            wt2r = ws_pool.tile([nj * HL, CHUNK], F32R, tag=f"wt2r{qi}")
            nc.vector.tensor_scalar_mul(wt2r, wtp, -1.0)
            wt_tiles[qi] = wt2r

        def pass2(qi, fill=()):
            fill = list(fill)
            chunks = plan[qi]
            nj = len(chunks)
            qsl = slice(qi * CHUNK, (qi + 1) * CHUNK)
            # flatten this qi's wt rows for the rank-1 inject: matmul rhs
            # base partition must be one of {0, 32, 64}, so pack row r at
            # (partition 32*(r%3), column block r//3); single reused buffer
            nrow = nj * HL
            nblk = (NQ * HL + 2) // 3
            wt_f = wf_pool.tile([65, nblk, CHUNK], F32R, tag="wtf")
            wt2r = wt_tiles[qi]
            for rr in range(3):
                cnt = (nrow - rr + 2) // 3
                if cnt <= 0:
                    continue
                nc.sync.dma_start(wt_f[32 * rr:32 * rr + 1, :cnt, :],
                                  wt2r[rr::3, :])

            ubs = []
            for h in range(HL):
                up = u_ps.tile([128, CHUNK], F32, tag="up")
                steps = [(t, j, diag, kc)
                         for t, (j, diag) in enumerate(chunks)
                         for kc in range(4)]
                nstep = len(steps)

                # software pipeline: PV matmuls lag the score/inject stream by
                # LAG steps so the PE never stalls on the Act-engine exp
                LAG = 4
                pend = []

                def emit_pv(idx, item):
                    j, kc, off, pp = item
                    _mm(nc, up[:, off:],
                        V[:, j * 4 + kc, (h // 2) * D:(h // 2 + 1) * D],
                        pp[:, off:], start=(idx == 0), stop=(idx == nstep - 1))

                for i, (t, j, diag, kc) in enumerate(steps):
                    k0 = j * CHUNK + kc * 128
                    off = kc * 128 if diag else 0
                    sp = ps_att.tile([128, CHUNK], F32, tag="ps")
                    _mm(nc, sp[:, off:], KT[:, h // 2, k0:k0 + 128],
                        QT[:, h, qi * CHUNK + off:(qi + 1) * CHUNK],
                        start=True, stop=False)
                    if diag:
                        _mm(nc, sp[:, off:off + 128], I128b, triT,
                            start=False, stop=False)
                    row = t * HL + h
                    rb = 32 * (row % 3)
                    _mm(nc, sp[:, off:], ones65[rb:rb + 1, :],
                        wt_f[rb:rb + 1, row // 3, off:],
                        start=False, stop=True)
                    pp = p2_pool.tile([128, CHUNK], BF16)
                    nc.scalar.activation(pp[:, off:], sp[:, off:], Act.Exp)
                    pend.append((i, (j, kc, off, pp)))
                    if len(pend) > LAG:
                        emit_pv(*pend.pop(0))
                for item in pend:
                    emit_pv(*item)
                ub = o2_pool.tile([128, CHUNK], BF16, tag=f"ub{h}",
                                  name=f"ub{h}_{qi}")
                nc.vector.tensor_copy(ub, up)
                ubs.append(ub)
                # PE-only filler (prev qi's output projection) between the
                # Act-bound h units
                nfill = 4 if h < HL - 1 else len(fill)
                for _ in range(min(nfill, len(fill))):
                    fill.pop(0)()

            return ubs

        def wo_unit(qi, ubs, mo):
            # one output-projection tile; ob copy split across Act and DVE
            qsl = slice(qi * CHUNK, (qi + 1) * CHUNK)
            po = ps_proj.tile([128, CHUNK], F32, tag="pp")
            for t in range(HL):
                _mm(nc, po, wo_sb[:, t, mo * 128:(mo + 1) * 128], ubs[t],
                    start=(t == 0), stop=(t == HL - 1))
            ob = o_pool.tile([128, CHUNK], BF16)
            nc.vector.tensor_copy(ob, po)
            nc.sync.dma_start(ap["outT"][mo * 128:(mo + 1) * 128, qsl], ob)

        # interleave: projections (PE-heavy) with pass-1 chains (Act/DVE-
        # heavy); the last pass-1 (the longest) is further interleaved with
        # the first pass-2s so its Act-engine burst hides under their PE work
        def pass1_all(qi):
            st = pass1_begin(qi)
            for t in range(st["nj"]):
                pass1_chunk(st, t)
            pass1_end(st)

        for sq in range(NQ - 1):
            proj_qk(sq)
            pass1_all(sq)
        proj_qk(NQ - 1)
        st3 = pass1_begin(NQ - 1)
        pass1_chunk(st3, 0)
        load_xt(0)
        proj_v(0)
        ubs0 = pass2(0)
        pass1_chunk(st3, 1)
        load_xt(1)
        proj_v(1)
        wo0 = [(lambda mo=mo: wo_unit(0, ubs0, mo)) for mo in range(HID // 128)]
        ubs1 = pass2(1, fill=wo0)
        pass1_chunk(st3, 2)
        load_xt(2)
        proj_v(2)
        pass1_chunk(st3, 3)
        load_xt(3)
        proj_v(3)
        pass1_end(st3)
        wo1 = [(lambda mo=mo: wo_unit(1, ubs1, mo)) for mo in range(HID // 128)]
        ubs2 = pass2(2, fill=wo1)
        wo2 = [(lambda mo=mo: wo_unit(2, ubs2, mo)) for mo in range(HID // 128)]
        ubs3 = pass2(3, fill=wo2)
        for mo in range(HID // 128):
            wo_unit(3, ubs3, mo)


def _build_program(plan):
    nc = bacc.Bacc("TRN2", target_bir_lowering=False, debug=False,
                   enable_asserts=False, num_devices=NCORES)
    ap = {}
    ap["hsT"] = nc.dram_tensor("hsT", [HID, S], BF16, kind="ExternalInput").ap()
    ap["wqk"] = nc.dram_tensor("wqk", [HID, (HL + KVL) * D], BF16, kind="ExternalInput").ap()
    ap["wv"] = nc.dram_tensor("wv", [HID, KVL * D], BF16, kind="ExternalInput").ap()
    ap["wo"] = nc.dram_tensor("wo", [HL * D, HID], BF16, kind="ExternalInput").ap()
    ap["bqk"] = nc.dram_tensor("bqk", [D, HL + KVL], F32, kind="ExternalInput").ap()
    ap["bv"] = nc.dram_tensor("bv", [1, KVL * D], F32R, kind="ExternalInput").ap()
    ap["cosT"] = nc.dram_tensor("cosT", [D, S], BF16, kind="ExternalInput").ap()
    ap["sinT"] = nc.dram_tensor("sinT", [D, S], BF16, kind="ExternalInput").ap()
    ap["rmat"] = nc.dram_tensor("rmat", [D, D], F32R, kind="ExternalInput").ap()
    ap["imat"] = nc.dram_tensor("imat", [128, 128], F32, kind="ExternalInput").ap()
    ap["imatb"] = nc.dram_tensor("imatb", [128, 128], BF16, kind="ExternalInput").ap()
    ap["triN"] = nc.dram_tensor("triN", [128, 128], BF16, kind="ExternalInput").ap()
    ap["triT"] = nc.dram_tensor("triT", [128, 128], BF16, kind="ExternalInput").ap()
    ap["ones1"] = nc.dram_tensor("ones1", [1, 128], F32R, kind="ExternalInput").ap()
    ap["ones65"] = nc.dram_tensor("ones65", [65, 128], F32R, kind="ExternalInput").ap()
    ap["outT"] = nc.dram_tensor("outT", [HID, S], BF16, kind="ExternalOutput").ap()

    with tile.TileContext(nc) as tc:
        _emit(tc, ap, plan)
    nc.compile()
    return nc


def _host_inputs(inputs):
    hs = np.asarray(inputs["hidden_states"], dtype=np.float32)
    Wq = np.asarray(inputs["Wq"], dtype=np.float32)
    bq = np.asarray(inputs["bq"], dtype=np.float32)
    Wk = np.asarray(inputs["Wk"], dtype=np.float32)
    bk = np.asarray(inputs["bk"], dtype=np.float32)
    Wv = np.asarray(inputs["Wv"], dtype=np.float32)
    bv_ = np.asarray(inputs["bv"], dtype=np.float32)
    Wo = np.asarray(inputs["Wo"], dtype=np.float32)

    cosT, sinT = _rope_tables()
    R = np.zeros((D, D), dtype=np.float32)
    R[64 + np.arange(64), np.arange(64)] = -1.0   # out[d'<64] = -q[d'+64]
    R[np.arange(64), 64 + np.arange(64)] = 1.0    # out[d'>=64] = q[d'-64]
    I = np.eye(128, dtype=np.float32)
    q = np.arange(128)
    triN = np.where(q[:, None] >= q[None, :], 0.0, NEG).astype(BFNP)
    triT = np.where(q[:, None] <= q[None, :], 0.0, NEG).astype(BFNP)

    Wq4 = (Wq * SCALE).reshape(HID, H, D)
    bq4 = (bq * SCALE).reshape(H, D)
    Wk4 = Wk.reshape(HID, HKV, D)
    bk4 = bk.reshape(HKV, D)
    Wv4 = Wv.reshape(HID, HKV, D)
    bv4 = bv_.reshape(HKV, D)
    Wo4 = Wo.reshape(H, D, HID)

    in_maps = []
    for c in range(NCORES):
        b, hg = divmod(c, NCORES // B)
        qh = slice(hg * HL, (hg + 1) * HL)
        kvh = slice(hg * KVL, (hg + 1) * KVL)
        wqk = np.concatenate([
            Wq4[:, qh].reshape(HID, HL * D),
            Wk4[:, kvh].reshape(HID, KVL * D)], axis=1)
        bqk = np.concatenate([bq4[qh], bk4[kvh]], axis=0).T  # [D, HL+KVL]
        in_maps.append({
            "hsT": hs[b].T.astype(BFNP),
            "wqk": wqk.astype(BFNP),
            "wv": Wv4[:, kvh].reshape(HID, KVL * D).astype(BFNP),
            "wo": Wo4[qh].reshape(HL * D, HID).astype(BFNP),
            "bqk": np.ascontiguousarray(bqk),
            "bv": bv4[kvh].reshape(1, KVL * D).copy(),
            "cosT": cosT.astype(BFNP),
            "sinT": sinT.astype(BFNP),
            "rmat": R,
            "imat": I,
            "imatb": I.astype(BFNP),
            "triN": triN,
            "triT": triT,
            "ones1": np.ones((1, 128), dtype=np.float32),
            "ones65": np.ones((65, 128), dtype=np.float32),
        })
    return in_maps


def get_program(inputs):
    am = np.asarray(inputs["attention_mask"], dtype=np.float32)
    plan = _classify_mask(am)
    key = str(plan)
    if key not in _CACHE:
        _CACHE[key] = _build_program(plan)
    return _CACHE[key], plan, None


def run(inputs, **spmd_kwargs):
    nc, plan, _ = get_program(inputs)
    in_maps = _host_inputs(inputs)
    res = run_bass_kernel_spmd(nc, in_maps, core_ids=list(range(NCORES)),
                               **spmd_kwargs)
    bo = np.asarray(inputs["bo"], dtype=np.float32)
    out = np.empty((B, S, HID), dtype=np.float32)
    gpb = NCORES // B
    for b in range(B):
        acc = np.zeros((HID, S), dtype=np.float32)
        for c in range(b * gpb, (b + 1) * gpb):
            acc += np.asarray(res.results[c]["outT"]).astype(np.float32)
        out[b] = acc.T + bo
    return out, res


def kernel(**inputs) -> np.ndarray:
    out, _ = run(inputs)
    return out


# revision 42
# speedup vs baseline: 1.0004x; 1.0004x over previous
"""Trainium2 Bass kernel for MemoryEfficientFlashAttention (B=2,S=2048,HID=2048,H=16,HKV=8,D=128,CHUNK=512).

Sharding: 8 cores = 2 batches x 4 head-groups (4 q heads / 2 kv heads per core).
Each core computes q/k/v projections (+RoPE), the chunked flash-attention
recurrence, and a row-sharded partial of the output projection (transposed).
Host sums the 4 partials per batch and adds bo.

Math: the reference's scan step is algebraically
    o_j = (o_{j-1} * e^{m_{j-1}} + Y_j) / (e^{m_{j-1}} + S_j)
with Y_j = exp(sc_j) @ V_j, S_j = rowsum exp(sc_j), m_j = running max.
Unrolled:  o_n = sum_j Y_j * C_{j-1} / (C_n * e^{m_n}),  C_j = prod_{l<=j} d_l,
    d_l = e^{m_{l-1}-m_l} + T_l,  T_l = rowsum exp(sc_l - m_l).
Pass 1 computes the (m, T, d, lnC) chains per row; pass 2 recomputes scores
transposed and accumulates  u = sum_j exp(sc_j^T + w_j - gamma) @ V  directly
in PSUM, with w_j = lnC_{j-1} and gamma = m_n + lnC_n (+ ln d_n if the
globally-last kv chunk was processed, reproducing the reference's final o/d
divide).  u is then exactly the final attention output; exponents are <= 0 so
everything is numerically stable.

Perf structure: bf16 operands for all large matmuls (full-rate at any moving
width), causal narrowing of the diagonal chunks (skip fully-masked k/q
sub-ranges), a single shared 128x128 triangular mask tile instead of
per-block mask DMA, single f32r rank-1 inject for the per-chunk log-scale
w, weights resident in SBUF (loaded once), and pass-1 (Act/DVE-heavy)
interleaved with the projections (PE-heavy).
"""

import os
import sys
from contextlib import ExitStack

import numpy as np
import ml_dtypes

sys.path.insert(0, "/opt/trn_rl_repo")
os.environ.setdefault("MYCRO_LOCAL_CACHE", "1")

import concourse.bass as bass  # noqa: E402
import concourse.tile as tile  # noqa: E402
from concourse import bacc, mybir  # noqa: E402
from concourse.bass_utils import run_bass_kernel_spmd  # noqa: E402

# Steer insert_act_table_loads to the table set that holds BOTH Exp and Ln
# (natural_log_exp_and_others) so the kernel loads one activation table
# instead of thrashing Exp<->Ln sets per query chunk. Indices into the
# act_info.json list are preserved; only the selection sees fewer options.
import collections  # noqa: E402
import concourse.hw_specs as _hw_specs  # noqa: E402

_gat_orig = _hw_specs.get_activation_tables


def _gat_combined(arch):
    tabs = _gat_orig(arch)
    both = {mybir.ActivationFunctionType.Exp, mybir.ActivationFunctionType.Ln}
    out = collections.OrderedDict()
    for name, s in tabs.items():
        if name == "natural_log_exp_and_others" or not (s & both):
            out[name] = s
        else:
            out[name] = s - both
    return out


bacc.get_activation_tables = _gat_combined

B, S, HID = 2, 2048, 2048
H, HKV, D = 16, 8, 128
CHUNK = 512
THETA = 1000000.0
NEG = -1e9
NCORES = 8
HL = H // (NCORES // B)      # 4 local q heads
KVL = HKV // (NCORES // B)   # 2 local kv heads
NQ = S // CHUNK              # 4 chunks
NT = HID // 128              # 16 hid tiles
SCALE = 1.0 / np.sqrt(np.float32(D))

F32 = mybir.dt.float32
F32R = mybir.dt.float32r
BF16 = mybir.dt.bfloat16
Alu = mybir.AluOpType
Act = mybir.ActivationFunctionType
BFNP = ml_dtypes.bfloat16

_CACHE = {}


def _rope_tables():
    inv_freq = 1.0 / (THETA ** (np.arange(0, D, 2, dtype=np.float32) / D))
    pos = np.arange(S, dtype=np.float32)
    freqs = pos[:, None].astype(np.float32) * inv_freq[None, :]
    emb = np.concatenate([freqs, freqs], axis=-1)  # [S, D]
    cosT = np.cos(emb).astype(np.float32).T.copy()
    sinT = np.sin(emb).astype(np.float32).T.copy()
    return cosT, sinT  # [D, S]


def _classify_mask(attention_mask):
    """Per (qi, j) CHUNKxCHUNK block: 'zero' | 'neg' | 'tri' (canonical causal
    diagonal), merged across batches so the SPMD program is identical on all
    cores. Only pure-causal masks are supported by this kernel."""
    q = np.arange(CHUNK)
    tri_full = np.where(q[:, None] >= q[None, :], 0.0, NEG).astype(np.float32)
    kinds = {}
    for qi in range(NQ):
        for j in range(NQ):
            kind = None
            for b in range(B):
                blk = attention_mask[b, 0, qi * CHUNK:(qi + 1) * CHUNK,
                                     j * CHUNK:(j + 1) * CHUNK]
                if np.all(blk == 0.0):
                    k = "zero"
                elif np.all(blk <= -1e6):
                    k = "neg"
                elif np.array_equal(blk, tri_full):
                    k = "tri"
                else:
                    raise NotImplementedError("non-causal mask block")
                if kind is None:
                    kind = k
                elif kind != k:
                    raise NotImplementedError("mask differs across batches")
            kinds[(qi, j)] = kind
    plan = {}
    for qi in range(NQ):
        processed = []
        for j in range(NQ):
            k = kinds[(qi, j)]
            if k == "neg" and len(processed) > 0:
                continue  # identity step under the reference's fp32 exp underflow
            assert k != "neg" or len(processed) == 0
            if k == "neg":
                # leading fully-masked chunk: contributes T=0 rows; unsupported
                raise NotImplementedError("leading all-neg chunk")
            processed.append((j, k == "tri"))
        plan[qi] = processed
    return plan


def _mm(nc, out, lhsT, rhs, start, stop):
    nc.tensor.matmul(out, lhsT, rhs, start=start, stop=stop)


def _emit(tc, ap, plan):
    nc = tc.nc

    with ExitStack() as top:
        # ---------------- persistent tensors ----------------
        pers = top.enter_context(tc.tile_pool(name="pers", bufs=1))
        QT = pers.tile([128, HL, S], BF16)             # rope'd q^T  [d, h, s]
        KT = pers.tile([128, KVL, S], BF16)            # rope'd k^T  [d, kv, s]
        V = pers.tile([128, S // 128, KVL * D], BF16)  # v natural [s_p, s_t, kv*d]
        xt_pool = top.enter_context(tc.tile_pool(name="xt", bufs=2))
        hsT_r = ap["hsT"].rearrange("(t p) s -> p t s", p=128)

        xts = {}

        def load_xt(sq):
            xt = xt_pool.tile([128, NT, CHUNK], BF16, tag="xt")
            ssl = slice(sq * CHUNK, (sq + 1) * CHUNK)
            for tq in range(4):
                nc.sync.dma_start(xt[:, tq * 4:(tq + 1) * 4, :],
                                  hsT_r[:, tq * 4:(tq + 1) * 4, ssl])
            xts[sq] = xt

        # startup DMAs ordered by first use: first-half weights + first x
        # chunk + rope tables first, everything else behind them
        wqk_sb = pers.tile([128, NT, (HL + KVL) * 128], BF16)
        wqk_r = ap["wqk"].rearrange("(t p) m -> p t m", p=128)
        ssl0 = slice(0, CHUNK)
        xt0 = xt_pool.tile([128, NT, CHUNK], BF16, tag="xt")
        xts[0] = xt0
        nc.sync.dma_start(wqk_sb[:, :2], wqk_r[:, :2])
        nc.sync.dma_start(xt0[:, :2, :], hsT_r[:, :2, ssl0])
        nc.sync.dma_start(wqk_sb[:, 2:4], wqk_r[:, 2:4])
        nc.sync.dma_start(xt0[:, 2:4, :], hsT_r[:, 2:4, ssl0])
        for tq in range(1, 4):
            nc.sync.dma_start(wqk_sb[:, tq * 4:(tq + 1) * 4],
                              wqk_r[:, tq * 4:(tq + 1) * 4])
            nc.sync.dma_start(xt0[:, tq * 4:(tq + 1) * 4, :],
                              hsT_r[:, tq * 4:(tq + 1) * 4, ssl0])
        cosT = pers.tile([128, S], BF16)
        sinT = pers.tile([128, S], BF16)
        nc.sync.dma_start(cosT[:, ssl0], ap["cosT"][:, ssl0])
        nc.sync.dma_start(sinT[:, ssl0], ap["sinT"][:, ssl0])
        R128 = pers.tile([128, 128], F32R)
        nc.sync.dma_start(R128, ap["rmat"])
        bqk = pers.tile([128, HL + KVL], F32)
        nc.sync.dma_start(bqk, ap["bqk"])
        for cq in range(1, NQ):
            cs = slice(cq * CHUNK, (cq + 1) * CHUNK)
            nc.sync.dma_start(cosT[:, cs], ap["cosT"][:, cs])
            nc.sync.dma_start(sinT[:, cs], ap["sinT"][:, cs])
        wv_sb = pers.tile([128, NT, KVL * D], BF16)
        nc.sync.dma_start(wv_sb, ap["wv"].rearrange("(t p) m -> p t m", p=128))
        bv = pers.tile([1, KVL * D], F32R)
        nc.sync.dma_start(bv, ap["bv"])
        ones1 = pers.tile([1, 128], F32R)
        nc.sync.dma_start(ones1, ap["ones1"])
        ones65 = pers.tile([65, 128], F32R)
        nc.sync.dma_start(ones65, ap["ones65"])
        I128f = pers.tile([128, 128], F32)
        nc.sync.dma_start(I128f, ap["imat"])
        I128b = pers.tile([128, 128], BF16)
        nc.sync.dma_start(I128b, ap["imatb"])
        triN = pers.tile([128, 128], BF16)
        nc.sync.dma_start(triN, ap["triN"])
        triT = pers.tile([128, 128], BF16)
        nc.sync.dma_start(triT, ap["triT"])
        wo_sb = pers.tile([128, HL, HID], BF16)
        wo_r = ap["wo"].rearrange("(t p) m -> p t m", p=128)
        for mo in range(4):
            nc.sync.dma_start(wo_sb[:, :, mo * 512:(mo + 1) * 512],
                              wo_r[:, :, mo * 512:(mo + 1) * 512])

        # ---------------- pools (single scope; PSUM budget = 8 banks) ------
        raw_pool = top.enter_context(tc.tile_pool(name="raw", bufs=2))
        t_pool = top.enter_context(tc.tile_pool(name="ropetmp", bufs=2))
        ps_proj = top.enter_context(tc.tile_pool(name="psproj", bufs=3, space="PSUM"))
        ps_att = top.enter_context(tc.tile_pool(name="psatt", bufs=4, space="PSUM"))
        u_ps = top.enter_context(tc.tile_pool(name="ups", bufs=1, space="PSUM"))

        ch_pool = top.enter_context(tc.tile_pool(name="chain", bufs=2))
        ws_pool = top.enter_context(tc.tile_pool(name="wstar", bufs=1))
        scr_pool = top.enter_context(tc.tile_pool(name="scratch", bufs=3))
        wt2_pool = top.enter_context(tc.tile_pool(name="wt2p", bufs=1))
        wf_pool = top.enter_context(tc.tile_pool(name="wflat", bufs=1))
        p2_pool = top.enter_context(tc.tile_pool(name="pprime", bufs=5))
        o2_pool = top.enter_context(tc.tile_pool(name="uout", bufs=2))
        o_pool = top.enter_context(tc.tile_pool(name="osb", bufs=4))

        wt_tiles = {}

        def proj_qk(sq):
            ssl = slice(sq * CHUNK, (sq + 1) * CHUNK)
            xt = xts.pop(sq)
            if sq + 1 < NQ:
                load_xt(sq + 1)

            # q^T and k^T projections, rope'd; the R-matmul + elementwise
            # rope tail run one m behind the qk accumulation so the PE never
            # waits on the Pool-engine bias add
            def rope_tail(m, raw):
                pr = ps_proj.tile([128, CHUNK], F32, tag="pp")
                _mm(nc, pr, R128, raw, start=True, stop=True)
                t1 = t_pool.tile([128, CHUNK], F32, tag="t1")
                nc.gpsimd.tensor_mul(t1, raw.bitcast(F32), cosT[:, ssl])
                t2 = t_pool.tile([128, CHUNK], F32, tag="t2")
                nc.vector.tensor_mul(t2, pr, sinT[:, ssl])
                dest = QT[:, m, ssl] if m < HL else KT[:, m - HL, ssl]
                nc.vector.tensor_add(dest, t1, t2)

            pend_rope = []
            for m in range(HL + KVL):
                ps = ps_proj.tile([128, CHUNK], F32, tag="pp")
                for t in range(NT):
                    _mm(nc, ps, wqk_sb[:, t, m * 128:(m + 1) * 128], xt[:, t],
                        start=(t == 0), stop=(t == NT - 1))
                raw = raw_pool.tile([128, CHUNK], F32R)
                nc.vector.tensor_scalar_add(raw, ps, bqk[:, m:m + 1])
                pend_rope.append((m, raw))
                if len(pend_rope) > 1:
                    rope_tail(*pend_rope.pop(0))
            for item in pend_rope:
                rope_tail(*item)

        def proj_v(sq):
            # v projection (natural layout), bias via K=1 matmul; runs late
            # (during the Act-bound attention phase) on a reloaded x chunk
            xt = xts.pop(sq)
            for ss in range(CHUNK // 128):
                pv = ps_proj.tile([128, CHUNK], F32, tag="pp")
                for t in range(NT):
                    _mm(nc, pv[:, :KVL * D], xt[:, t, ss * 128:(ss + 1) * 128], wv_sb[:, t],
                        start=(t == 0), stop=False)
                _mm(nc, pv[:, :KVL * D], ones1, bv, start=False, stop=True)
                nc.vector.tensor_copy(V[:, sq * 4 + ss, :], pv[:, :KVL * D])

        def pass1_begin(qi):
            chunks = plan[qi]  # list of (j, is_diag)
            nj = len(chunks)
            # nmstack[:, t, :] = running max BEFORE chunk t (+m domain)
            nms = ws_pool.tile([128, nj + 1, HL * 4], F32, tag=f"nms{qi}")
            nc.vector.memset(nms[:, 0, :], -1e30)
            sraw = ws_pool.tile([128, nj, HL * 4], F32, tag=f"sr{qi}")
            dstore = ws_pool.tile([128, nj, HL * 4], F32, tag=f"ds{qi}")
            return {"qi": qi, "chunks": chunks, "nj": nj, "nms": nms,
                    "sraw": sraw, "dstore": dstore}

        # ---- running max + exp-sum chains (one chunk) ----
        # scores are O(6) here, so exp(sc) cannot overflow: accumulate
        # raw sums S_raw = sum exp(sc) on the Act engine (decoupled from
        # the running-max chain) and rescale T = S_raw * e^{-m} after.
        def pass1_chunk(st, t):
            qi, nms, sraw = st["qi"], st["nms"], st["sraw"]
            for tt, (j, diag) in enumerate(st["chunks"]):
                if tt != t:
                    continue
                k0 = j * CHUNK
                mxe = ch_pool.tile([128, HL * 4], F32, tag="mxe")
                lnmx = ch_pool.tile([128, HL * 4], F32, tag="lnmx")
                for h in range(HL):
                    for sub in range(4):
                        col = h * 4 + sub
                        q0 = qi * CHUNK + sub * 128
                        w = (sub + 1) * 128 if diag else CHUNK
                        ps = ps_att.tile([128, CHUNK], F32, tag="ps")
                        _mm(nc, ps[:, :w], QT[:, h, q0:q0 + 128],
                            KT[:, h // 2, k0:k0 + w],
                            start=True, stop=not diag)
                        if diag:
                            _mm(nc, ps[:, w - 128:w], I128b, triN,
                                start=False, stop=True)
                        scr2 = scr_pool.tile([128, CHUNK], BF16, tag="exp_out")
                        nc.scalar.activation(
                            scr2[:, :w], ps[:, :w], Act.Exp,
                            accum_out=sraw[:, t, col:col + 1])
                        # row max from the bf16 exp output: 2x DVE mode, and
                        # m = ln(max exp(sc)) recovers the running max
                        nc.vector.tensor_reduce(
                            mxe[:, col:col + 1], scr2[:, :w],
                            axis=mybir.AxisListType.X, op=Alu.max)
                nc.scalar.activation(lnmx, mxe, Act.Ln)
                nc.vector.tensor_tensor(nms[:, t + 1, :], nms[:, t, :],
                                        lnmx, Alu.max)

        def pass1_end(st):
            qi, nj, nms, chunks = st["qi"], st["nj"], st["nms"], st["chunks"]
            sraw, dstore = st["sraw"], st["dstore"]
            lnq = ws_pool.tile([128, nj, HL * 4], F32, tag=f"ln{qi}")
            Wadj = ws_pool.tile([128, nj, HL * 4], F32, tag=f"wa{qi}")
            # batched chain tail: T = S_raw * e^{-m_new}, d = e^{m_old-m_new}+T
            dm = ch_pool.tile([128, nj, HL * 4], F32, tag="dmall")
            nc.vector.tensor_sub(dm, nms[:, :nj, :], nms[:, 1:, :])
            pj = ch_pool.tile([128, nj, HL * 4], F32, tag="pjall")
            nc.scalar.activation(pj, dm, Act.Exp)
            emn = ch_pool.tile([128, nj, HL * 4], F32, tag="emnall")
            nc.scalar.activation(emn, nms[:, 1:, :], Act.Exp, scale=-1.0)
            nc.vector.tensor_mul(dstore, sraw, emn)
            nc.vector.tensor_add(dstore, dstore, pj)
            m_fin = nms[:, nj, :]
            # inject_t = -m_n - ln(prod_{l>=t} d_l * d_n^flag): backward
            # products then ONE batched Ln (avoids Exp<->Ln table thrash)
            if any(j == NQ - 1 for (j, _) in chunks):
                nc.vector.tensor_mul(dstore[:, nj - 1, :],
                                     dstore[:, nj - 1, :],
                                     dstore[:, nj - 1, :])
            for t in range(nj - 2, -1, -1):
                nc.vector.tensor_mul(dstore[:, t, :], dstore[:, t, :],
                                     dstore[:, t + 1, :])
            nc.scalar.activation(lnq, dstore, Act.Ln)
            for t in range(nj):
                nc.vector.tensor_add(Wadj[:, t, :], m_fin, lnq[:, t, :])

            # transpose Wadj -> wt2 [nj*HL, 512] (row = (t, h), col = sq),
            # then flatten rows onto partition 0 (matmul rhs needs base
            # partition 0) as f32r for the single rank-1 inject
            wtp = ps_att.tile([nj * HL, 4, 128], F32, tag="ps", name=f"wtp{qi}")
            wadj_r = Wadj.rearrange("p n (x a) -> p n x a", a=4)
            for sub in range(4):
                nc.tensor.transpose(wtp[:, sub, :], wadj_r[:, :, :, sub], I128f)
            wt2r = ws_pool.tile([nj * HL, CHUNK], F32R, tag=f"wt2r{qi}")
            nc.vector.tensor_scalar_mul(wt2r, wtp, -1.0)
            wt_tiles[qi] = wt2r

        def pass2(qi, fill=()):
            fill = list(fill)
            chunks = plan[qi]
            nj = len(chunks)
            qsl = slice(qi * CHUNK, (qi + 1) * CHUNK)
            # flatten this qi's wt rows for the rank-1 inject: matmul rhs
            # base partition must be one of {0, 32, 64}, so pack row r at
            # (partition 32*(r%3), column block r//3); single reused buffer
            nrow = nj * HL
            nblk = (NQ * HL + 2) // 3
            wt_f = wf_pool.tile([65, nblk, CHUNK], F32R, tag="wtf")
            wt2r = wt_tiles[qi]
            for rr in range(3):
                cnt = (nrow - rr + 2) // 3
                if cnt <= 0:
                    continue
                nc.sync.dma_start(wt_f[32 * rr:32 * rr + 1, :cnt, :],
                                  wt2r[rr::3, :])

            ubs = []
            for h in range(HL):
                up = u_ps.tile([128, CHUNK], F32, tag="up")
                steps = [(t, j, diag, kc)
                         for t, (j, diag) in enumerate(chunks)
                         for kc in range(4)]
                nstep = len(steps)

                # software pipeline: PV matmuls lag the score/inject stream by
                # LAG steps so the PE never stalls on the Act-engine exp
                LAG = 4
                pend = []

                def emit_pv(idx, item):
                    j, kc, off, pp = item
                    _mm(nc, up[:, off:],
                        V[:, j * 4 + kc, (h // 2) * D:(h // 2 + 1) * D],
                        pp[:, off:], start=(idx == 0), stop=(idx == nstep - 1))

                for i, (t, j, diag, kc) in enumerate(steps):
                    k0 = j * CHUNK + kc * 128
                    off = kc * 128 if diag else 0
                    sp = ps_att.tile([128, CHUNK], F32, tag="ps")
                    _mm(nc, sp[:, off:], KT[:, h // 2, k0:k0 + 128],
                        QT[:, h, qi * CHUNK + off:(qi + 1) * CHUNK],
                        start=True, stop=False)
                    if diag:
                        _mm(nc, sp[:, off:off + 128], I128b, triT,
                            start=False, stop=False)
                    row = t * HL + h
                    rb = 32 * (row % 3)
                    _mm(nc, sp[:, off:], ones65[rb:rb + 1, :],
                        wt_f[rb:rb + 1, row // 3, off:],
                        start=False, stop=True)
                    pp = p2_pool.tile([128, CHUNK], BF16)
                    nc.scalar.activation(pp[:, off:], sp[:, off:], Act.Exp)
                    pend.append((i, (j, kc, off, pp)))
                    if len(pend) > LAG:
                        emit_pv(*pend.pop(0))
                for item in pend:
                    emit_pv(*item)
                ub = o2_pool.tile([128, CHUNK], BF16, tag=f"ub{h}",
                                  name=f"ub{h}_{qi}")
                nc.vector.tensor_copy(ub, up)
                ubs.append(ub)
                # PE-only filler (prev qi's output projection) between the
                # Act-bound h units
                nfill = 3 if h < HL - 1 else len(fill)
                for _ in range(min(nfill, len(fill))):
                    fill.pop(0)()

            return ubs

        def wo_unit(qi, ubs, mo):
            # one output-projection tile; ob copy split across Act and DVE
            qsl = slice(qi * CHUNK, (qi + 1) * CHUNK)
            po = ps_proj.tile([128, CHUNK], F32, tag="pp")
            for t in range(HL):
                _mm(nc, po, wo_sb[:, t, mo * 128:(mo + 1) * 128], ubs[t],
                    start=(t == 0), stop=(t == HL - 1))
            ob = o_pool.tile([128, CHUNK], BF16)
            nc.vector.tensor_copy(ob, po)
            nc.sync.dma_start(ap["outT"][mo * 128:(mo + 1) * 128, qsl], ob)

        # interleave: projections (PE-heavy) with pass-1 chains (Act/DVE-
        # heavy); the last pass-1 (the longest) is further interleaved with
        # the first pass-2s so its Act-engine burst hides under their PE work
        def pass1_all(qi):
            st = pass1_begin(qi)
            for t in range(st["nj"]):
                pass1_chunk(st, t)
            pass1_end(st)

        for sq in range(NQ - 1):
            proj_qk(sq)
            pass1_all(sq)
        proj_qk(NQ - 1)
        st3 = pass1_begin(NQ - 1)
        pass1_chunk(st3, 0)
        load_xt(0)
        proj_v(0)
        ubs0 = pass2(0)
        pass1_chunk(st3, 1)
        load_xt(1)
        proj_v(1)
        wo0 = [(lambda mo=mo: wo_unit(0, ubs0, mo)) for mo in range(HID // 128)]
        ubs1 = pass2(1, fill=wo0)
        pass1_chunk(st3, 2)
        load_xt(2)
        proj_v(2)
        pass1_chunk(st3, 3)
        load_xt(3)
        proj_v(3)
        pass1_end(st3)
        wo1 = [(lambda mo=mo: wo_unit(1, ubs1, mo)) for mo in range(HID // 128)]
        ubs2 = pass2(2, fill=wo1)
        wo2 = [(lambda mo=mo: wo_unit(2, ubs2, mo)) for mo in range(HID // 128)]
        ubs3 = pass2(3, fill=wo2)
        for mo in range(HID // 128):
            wo_unit(3, ubs3, mo)


def _build_program(plan):
    nc = bacc.Bacc("TRN2", target_bir_lowering=False, debug=False,
                   enable_asserts=False, num_devices=NCORES)
    ap = {}
    ap["hsT"] = nc.dram_tensor("hsT", [HID, S], BF16, kind="ExternalInput").ap()
    ap["wqk"] = nc.dram_tensor("wqk", [HID, (HL + KVL) * D], BF16, kind="ExternalInput").ap()
    ap["wv"] = nc.dram_tensor("wv", [HID, KVL * D], BF16, kind="ExternalInput").ap()
    ap["wo"] = nc.dram_tensor("wo", [HL * D, HID], BF16, kind="ExternalInput").ap()
    ap["bqk"] = nc.dram_tensor("bqk", [D, HL + KVL], F32, kind="ExternalInput").ap()
    ap["bv"] = nc.dram_tensor("bv", [1, KVL * D], F32R, kind="ExternalInput").ap()
    ap["cosT"] = nc.dram_tensor("cosT", [D, S], BF16, kind="ExternalInput").ap()
    ap["sinT"] = nc.dram_tensor("sinT", [D, S], BF16, kind="ExternalInput").ap()
    ap["rmat"] = nc.dram_tensor("rmat", [D, D], F32R, kind="ExternalInput").ap()
    ap["imat"] = nc.dram_tensor("imat", [128, 128], F32, kind="ExternalInput").ap()
    ap["imatb"] = nc.dram_tensor("imatb", [128, 128], BF16, kind="ExternalInput").ap()
    ap["triN"] = nc.dram_tensor("triN", [128, 128], BF16, kind="ExternalInput").ap()
    ap["triT"] = nc.dram_tensor("triT", [128, 128], BF16, kind="ExternalInput").ap()
    ap["ones1"] = nc.dram_tensor("ones1", [1, 128], F32R, kind="ExternalInput").ap()
    ap["ones65"] = nc.dram_tensor("ones65", [65, 128], F32R, kind="ExternalInput").ap()
    ap["outT"] = nc.dram_tensor("outT", [HID, S], BF16, kind="ExternalOutput").ap()

    with tile.TileContext(nc) as tc:
        _emit(tc, ap, plan)
    nc.compile()
    return nc


def _host_inputs(inputs):
    hs = np.asarray(inputs["hidden_states"], dtype=np.float32)
    Wq = np.asarray(inputs["Wq"], dtype=np.float32)
    bq = np.asarray(inputs["bq"], dtype=np.float32)
    Wk = np.asarray(inputs["Wk"], dtype=np.float32)
    bk = np.asarray(inputs["bk"], dtype=np.float32)
    Wv = np.asarray(inputs["Wv"], dtype=np.float32)
    bv_ = np.asarray(inputs["bv"], dtype=np.float32)
    Wo = np.asarray(inputs["Wo"], dtype=np.float32)

    cosT, sinT = _rope_tables()
    R = np.zeros((D, D), dtype=np.float32)
    R[64 + np.arange(64), np.arange(64)] = -1.0   # out[d'<64] = -q[d'+64]
    R[np.arange(64), 64 + np.arange(64)] = 1.0    # out[d'>=64] = q[d'-64]
    I = np.eye(128, dtype=np.float32)
    q = np.arange(128)
    triN = np.where(q[:, None] >= q[None, :], 0.0, NEG).astype(BFNP)
    triT = np.where(q[:, None] <= q[None, :], 0.0, NEG).astype(BFNP)

    Wq4 = (Wq * SCALE).reshape(HID, H, D)
    bq4 = (bq * SCALE).reshape(H, D)
    Wk4 = Wk.reshape(HID, HKV, D)
    bk4 = bk.reshape(HKV, D)
    Wv4 = Wv.reshape(HID, HKV, D)
    bv4 = bv_.reshape(HKV, D)
    Wo4 = Wo.reshape(H, D, HID)

    in_maps = []
    for c in range(NCORES):
        b, hg = divmod(c, NCORES // B)
        qh = slice(hg * HL, (hg + 1) * HL)
        kvh = slice(hg * KVL, (hg + 1) * KVL)
        wqk = np.concatenate([
            Wq4[:, qh].reshape(HID, HL * D),
            Wk4[:, kvh].reshape(HID, KVL * D)], axis=1)
        bqk = np.concatenate([bq4[qh], bk4[kvh]], axis=0).T  # [D, HL+KVL]
        in_maps.append({
            "hsT": hs[b].T.astype(BFNP),
            "wqk": wqk.astype(BFNP),
            "wv": Wv4[:, kvh].reshape(HID, KVL * D).astype(BFNP),
            "wo": Wo4[qh].reshape(HL * D, HID).astype(BFNP),
            "bqk": np.ascontiguousarray(bqk),
            "bv": bv4[kvh].reshape(1, KVL * D).copy(),
            "cosT": cosT.astype(BFNP),
            "sinT": sinT.astype(BFNP),
            "rmat": R,
            "imat": I,
            "imatb": I.astype(BFNP),
            "triN": triN,
            "triT": triT,
            "ones1": np.ones((1, 128), dtype=np.float32),
            "ones65": np.ones((65, 128), dtype=np.float32),
        })
    return in_maps


def get_program(inputs):
    am = np.asarray(inputs["attention_mask"], dtype=np.float32)
    plan = _classify_mask(am)
    key = str(plan)
    if key not in _CACHE:
        _CACHE[key] = _build_program(plan)
    return _CACHE[key], plan, None


def run(inputs, **spmd_kwargs):
    nc, plan, _ = get_program(inputs)
    in_maps = _host_inputs(inputs)
    res = run_bass_kernel_spmd(nc, in_maps, core_ids=list(range(NCORES)),
                               **spmd_kwargs)
    bo = np.asarray(inputs["bo"], dtype=np.float32)
    out = np.empty((B, S, HID), dtype=np.float32)
    gpb = NCORES // B
    for b in range(B):
        acc = np.zeros((HID, S), dtype=np.float32)
        for c in range(b * gpb, (b + 1) * gpb):
            acc += np.asarray(res.results[c]["outT"]).astype(np.float32)
        out[b] = acc.T + bo
    return out, res


def kernel(**inputs) -> np.ndarray:
    out, _ = run(inputs)
    return out
